# revision 8
# baseline (speedup 1.0000x reference)
"""Trainium2 Bass kernel for sparse 3D conv (gather -> 8x[32,32] GEMM -> scatter-add).

Design: tokens are sorted by output voxel id and partitioned across 8 cores by
contiguous output-row ranges, so each output row is owned by exactly one core
(no cross-core reduction, no RMW scatter). Per core, tokens are packed into
128-token tiles that never split a segment. On device, each tile does:

  P8 = X_tile^T @ W_all          one matmul, [128 tok, 8*32] for all 8 kernel offsets
  P  = reduce_k(onehot(k) * P8)  per-token offset select (DVE)
  S  = (iota == segcode)         one-hot token->segment matrix (DVE)
  Y  = S^T @ P                   segment sums, [128 seg-slots, 32] (matmul, PSUM)

Y rows are written densely per tile (unique rows, single write). Wire traffic is
minimized because the axon tunnel is the bottleneck (~55 MB/s incompressible):
x is uploaded as int8 with a global scale folded into the weights, y is
downloaded as int8 with per-row fp16 scales computed on device.
"""

import sys

sys.path.insert(0, "/opt/trn_rl_repo")

import numpy as np

import concourse.bacc as bacc
import concourse.mybir as mybir
import concourse.tile as tile

P = 128  # tokens per tile == segment slots per tile
C = 32
K = 8
N_CORES = 8
BATCH = 16  # tiles per DMA group
XQ = 126.0  # int8 quant ranges (margin below 127 to avoid saturation)
YQ = 126.0


def host_prepare(x, weight, offset_idx, out_idx):
    x = np.asarray(x, np.float32)
    w = np.asarray(weight, np.float32)
    ks_full = np.asarray(offset_idx, np.int64)
    oi_full = np.asarray(out_idx, np.int64)
    N = x.shape[0]
    M = int(oi_full.max()) + 1
    Sc = -(-M // N_CORES)  # output rows per core

    order = np.argsort(oi_full, kind="stable")
    souts = oi_full[order]
    cb = np.searchsorted(souts, np.arange(N_CORES + 1) * Sc)

    import ml_dtypes

    wall = np.transpose(w, (1, 0, 2)).reshape(C, K * C).astype(np.float32)
    xq_full = x.astype(ml_dtypes.bfloat16)

    percore = []
    for c in range(N_CORES):
        t0, t1 = int(cb[c]), int(cb[c + 1])
        ids = order[t0:t1]
        lseg = souts[t0:t1] - c * Sc
        kk = ks_full[ids]
        T = t1 - t0
        if T == 0:
            percore.append(None)
            continue
        is_start = np.r_[True, lseg[1:] != lseg[:-1]]
        B = np.flatnonzero(is_start)
        breaks = [0]
        while T - breaks[-1] > P:
            j = np.searchsorted(B, breaks[-1] + P, side="right") - 1
            nxt = int(B[j])
            assert nxt > breaks[-1], "segment longer than 128 tokens"
            breaks.append(nxt)
        breaks.append(T)
        breaks = np.asarray(breaks)
        nb = len(breaks) - 1
        tix = np.searchsorted(breaks, np.arange(T), side="right") - 1
        slot = np.arange(T) - breaks[tix]
        segfirst = lseg[breaks[:-1]]
        code = lseg - segfirst[tix]
        assert code.max() < P
        percore.append(
            {
                "ids": ids,
                "tix": tix,
                "slot": slot,
                "code": code,
                "kk": kk,
                "nb": nb,
                "segfirst": segfirst,
                "nseg": int(lseg[-1]) + 1,
            }
        )

    nt = max(pc["nb"] for pc in percore if pc is not None)
    cores = []
    metas = []
    for c in range(N_CORES):
        pc = percore[c]
        xs = np.zeros((nt * P, C), ml_dtypes.bfloat16)
        codes = np.full((P, nt), 255, np.int16)
        kcodes = np.zeros((P, nt), np.int16)
        if pc is not None:
            g = pc["tix"] * P + pc["slot"]
            xs[g] = xq_full[pc["ids"]]
            codes[pc["slot"], pc["tix"]] = pc["code"]
            kcodes[pc["slot"], pc["tix"]] = pc["kk"]
        cores.append(
            {
                "xs": np.ascontiguousarray(xs.T),
                "codes": codes,
                "kcodes": kcodes,
                "wall": wall,
            }
        )
        metas.append(
            None
            if pc is None
            else {"segfirst": pc["segfirst"], "nseg": pc["nseg"]}
        )
    meta = {"nt": nt, "M": M, "Sc": Sc, "core_meta": metas}
    return cores, meta


def build_bass(meta):
    nt = meta["nt"]
    nc = bacc.Bacc("TRN2")
    xs = nc.dram_tensor("xs", [C, nt * P], mybir.dt.bfloat16, kind="ExternalInput")
    codes = nc.dram_tensor("codes", [P, nt], mybir.dt.int16, kind="ExternalInput")
    kcodes = nc.dram_tensor("kcodes", [P, nt], mybir.dt.int16, kind="ExternalInput")
    wall = nc.dram_tensor("wall", [C, K * C], mybir.dt.float32, kind="ExternalInput")
    yq = nc.dram_tensor("yq", [P, nt * C], mybir.dt.int8, kind="ExternalOutput")
    ysc = nc.dram_tensor("ysc", [P, nt], mybir.dt.float16, kind="ExternalOutput")

    eq = mybir.AluOpType.is_equal
    mul = mybir.AluOpType.mult

    with tile.TileContext(nc) as tc:
        with (
            tc.tile_pool(name="const", bufs=1) as constp,
            tc.tile_pool(name="xb", bufs=2) as xbp,
            tc.tile_pool(name="ct", bufs=2) as ctp,
            tc.tile_pool(name="kt", bufs=2) as ktp,
            tc.tile_pool(name="pm", bufs=3) as pmp,
            tc.tile_pool(name="pf", bufs=3) as pfp,
            tc.tile_pool(name="pb", bufs=3) as pbp,
            tc.tile_pool(name="s", bufs=3) as sp,
            tc.tile_pool(name="r", bufs=3) as rp,
            tc.tile_pool(name="ysg", bufs=2) as ysgp,
            tc.tile_pool(name="ssg", bufs=2) as ssgp,
            tc.tile_pool(name="p8", bufs=2, space="PSUM") as p8p,
            tc.tile_pool(name="py", bufs=2, space="PSUM") as pyp,
        ):
            iota_s = constp.tile([P, P], mybir.dt.int16, tag="iota_s")
            nc.gpsimd.iota(iota_s[:], [[1, P]], channel_multiplier=0)
            iota_k = constp.tile([P, K * C], mybir.dt.int16, tag="iota_k")
            nc.gpsimd.iota(iota_k[:], [[1, K], [0, C]], channel_multiplier=0)
            wf = constp.tile([C, K * C], mybir.dt.float32, tag="wf")
            nc.sync.dma_start(out=wf[:], in_=wall[:, :])
            wsb = constp.tile([C, K * C], mybir.dt.bfloat16, tag="wsb")
            nc.vector.tensor_copy(out=wsb[:], in_=wf[:])

            nbatch = -(-nt // BATCH)
            for b in range(nbatch):
                j0 = b * BATCH
                nbt = min(BATCH, nt - j0)
                xbt = xbp.tile([C, nbt * P], mybir.dt.bfloat16, tag="xb")
                nc.sync.dma_start(out=xbt[:], in_=xs[:, j0 * P : (j0 + nbt) * P])
                cti = ctp.tile([P, nbt], mybir.dt.int16, tag="cti")
                nc.sync.dma_start(out=cti[:], in_=codes[:, j0 : j0 + nbt])
                ct = ctp.tile([P, nbt], mybir.dt.float32, tag="ct")
                nc.vector.tensor_copy(out=ct[:], in_=cti[:])
                kti = ktp.tile([P, nbt], mybir.dt.int16, tag="kti")
                nc.sync.dma_start(out=kti[:], in_=kcodes[:, j0 : j0 + nbt])
                kt = ktp.tile([P, nbt], mybir.dt.float32, tag="kt")
                nc.vector.tensor_copy(out=kt[:], in_=kti[:])
                ysg = ysgp.tile([P, nbt * C], mybir.dt.int8, tag="ysg")
                ssg = ssgp.tile([P, nbt], mybir.dt.float16, tag="ssg")
                for j in range(nbt):
                    p8 = p8p.tile([P, K * C], mybir.dt.float32, tag="p8")
                    nc.tensor.matmul(
                        out=p8[:],
                        lhsT=xbt[:, j * P : (j + 1) * P],
                        rhs=wsb[:],
                        start=True,
                        stop=True,
                    )
                    pm = pmp.tile([P, K * C], mybir.dt.bfloat16, tag="pm")
                    nc.vector.scalar_tensor_tensor(
                        out=pm[:],
                        in0=iota_k[:],
                        scalar=kt[:, j : j + 1],
                        in1=p8[:],
                        op0=eq,
                        op1=mul,
                    )
                    pf = pfp.tile([P, C], mybir.dt.float32, tag="pf")
                    nc.vector.tensor_reduce(
                        out=pf[:],
                        in_=pm[:].rearrange("p (k c) -> p c k", k=K),
                        axis=mybir.AxisListType.X,
                        op=mybir.AluOpType.add,
                    )
                    pb = pbp.tile([P, C], mybir.dt.bfloat16, tag="pb")
                    nc.vector.tensor_copy(out=pb[:], in_=pf[:])
                    s = sp.tile([P, P], mybir.dt.bfloat16, tag="s")
                    nc.gpsimd.tensor_scalar(
                        out=s[:],
                        in0=iota_s[:],
                        scalar1=ct[:, j : j + 1],
                        scalar2=None,
                        op0=eq,
                    )
                    y = pyp.tile([P, C], mybir.dt.float32, tag="y")
                    nc.tensor.matmul(
                        out=y[:], lhsT=s[:], rhs=pb[:], start=True, stop=True
                    )
                    rmax = rp.tile([P, 1], mybir.dt.float32, tag="rmax")
                    nc.vector.tensor_reduce(
                        out=rmax[:],
                        in_=y[:],
                        axis=mybir.AxisListType.X,
                        op=mybir.AluOpType.max,
                        apply_absolute_value=True,
                    )
                    rg = rp.tile([P, 1], mybir.dt.float32, tag="rg")
                    nc.vector.tensor_scalar_max(out=rg[:], in0=rmax[:], scalar1=1e-6)
                    inv = rp.tile([P, 1], mybir.dt.float32, tag="inv")
                    nc.vector.reciprocal(out=inv[:], in_=rg[:])
                    nc.vector.tensor_scalar(
                        out=ysg[:, j * C : (j + 1) * C],
                        in0=y[:],
                        scalar1=inv[:, 0:1],
                        scalar2=YQ,
                        op0=mul,
                        op1=mul,
                    )
                    nc.vector.tensor_scalar_mul(
                        out=ssg[:, j : j + 1], in0=rg[:], scalar1=1.0 / YQ
                    )
                nc.sync.dma_start(
                    out=yq[:, j0 * C : (j0 + nbt) * C], in_=ysg[:]
                )
                nc.sync.dma_start(out=ysc[:, j0 : j0 + nbt], in_=ssg[:])
    nc.compile()
    return nc


def assemble(results, meta, num_out):
    nt = meta["nt"]
    Sc = meta["Sc"]
    y = np.zeros((num_out, C), np.float32)
    for c in range(N_CORES):
        cm = meta["core_meta"][c]
        if cm is None:
            continue
        yqc = np.asarray(results[c]["yq"]).reshape(P, nt, C)
        yscc = np.asarray(results[c]["ysc"], np.float32)
        segfirst = cm["segfirst"]
        nseg = cm["nseg"]
        s = np.arange(nseg)
        tau = np.searchsorted(segfirst, s, side="right") - 1
        r = s - segfirst[tau]
        y[c * Sc : c * Sc + nseg] = (
            yqc[r, tau, :].astype(np.float32) * yscc[r, tau][:, None]
        )
    return y


def kernel(x, weight, offset_idx, out_idx, num_out):
    from concourse.bass_utils import run_bass_kernel_spmd

    num_out = int(num_out)
    cores, meta = host_prepare(x, weight, offset_idx, out_idx)
    nc = build_bass(meta)
    res = run_bass_kernel_spmd(nc, cores, core_ids=list(range(N_CORES)))
    return assemble(res.results, meta, num_out)


def _sim_test():
    """Small-scale correctness check in CoreSim (no hardware)."""
    from concourse.bass_interp import CoreSim

    rng = np.random.default_rng(0)
    N, GRID = 20000, 64
    coords = rng.integers(0, GRID, size=(N, 3))
    off = coords % 2
    offset_idx = (off[:, 0] * 4 + off[:, 1] * 2 + off[:, 2]).astype(np.int64)
    oc = coords // 2
    flat = (oc[:, 0] * (GRID // 2) + oc[:, 1]) * (GRID // 2) + oc[:, 2]
    _, inv = np.unique(flat, return_inverse=True)
    out_idx = inv.astype(np.int64)
    x = rng.standard_normal((N, C), np.float32)
    w = (rng.standard_normal((K, C, C)) * 0.1).astype(np.float32)

    M = int(out_idx.max()) + 1
    expected = np.zeros((M, C), np.float32)
    for k in range(K):
        sel = offset_idx == k
        np.add.at(expected, out_idx[sel], x[sel] @ w[k])

    cores, meta = host_prepare(x, w, offset_idx, out_idx)
    print(f"sim test: N={N} M={M} nt={meta['nt']}")
    nc = build_bass(meta)
    results = []
    for c in range(N_CORES):
        sim = CoreSim(nc, require_finite=False, require_nnan=False)
        for name, arr in cores[c].items():
            sim.tensor(name)[:] = arr
        sim.simulate(check_with_hw=False)
        results.append({"yq": sim.tensor("yq").copy(), "ysc": sim.tensor("ysc").copy()})
        print(f"core {c} simulated")
    actual = assemble(results, meta, M)
    denom = np.abs(expected).max()
    rel = np.abs(actual - expected).max() / denom
    print(f"sim relative error: {rel:.3e}")
    assert rel < 2e-2, rel


if __name__ == "__main__":
    _sim_test()


# revision 14
# speedup vs baseline: 1.1418x; 1.1418x over previous
"""Trainium2 Bass kernel for sparse 3D conv (gather -> 8x[32,32] GEMM -> scatter-add).

Design: tokens are sorted by output voxel id and partitioned across 8 cores by
contiguous output-row ranges, so each output row is owned by exactly one core
(no cross-core reduction, no RMW scatter). Per core, tokens are packed into
128-token tiles that never split a segment. On device, each tile does:

  P8 = X_tile^T @ W_all          one matmul, [128 tok, 8*32] for all 8 kernel offsets
  P  = reduce_k(onehot(k) * P8)  per-token offset select (DVE)
  S  = (iota == segcode)         one-hot token->segment matrix (DVE)
  Y  = S^T @ P                   segment sums, [128 seg-slots, 32] (matmul, PSUM)

Y rows are written densely per tile (unique rows, single write). Wire traffic is
minimized because the axon tunnel is the bottleneck (~55 MB/s incompressible):
x is uploaded as int8 with a global scale folded into the weights, y is
downloaded as int8 with per-row fp16 scales computed on device.
"""

import sys

sys.path.insert(0, "/opt/trn_rl_repo")

import numpy as np

import concourse.bacc as bacc
import concourse.mybir as mybir
import concourse.tile as tile

P = 128  # tokens per tile == segment slots per tile
C = 32
K = 8
N_CORES = 8
BATCH = 16  # tiles per DMA group
XQ = 126.0  # int8 quant ranges (margin below 127 to avoid saturation)
YQ = 126.0


def host_prepare(x, weight, offset_idx, out_idx):
    x = np.asarray(x, np.float32)
    w = np.asarray(weight, np.float32)
    ks_full = np.asarray(offset_idx, np.int64)
    oi_full = np.asarray(out_idx, np.int64)
    N = x.shape[0]
    M = int(oi_full.max()) + 1
    Sc = -(-M // N_CORES)  # output rows per core

    order = np.argsort(oi_full, kind="stable")
    souts = oi_full[order]
    cb = np.searchsorted(souts, np.arange(N_CORES + 1) * Sc)

    wall = np.transpose(w, (1, 0, 2)).reshape(C, K * C).astype(np.float32)
    # per-token int8 quantization: x[i] ~= xq[i] * xsc[i]
    rmax = np.maximum(np.abs(x).max(axis=1), 1e-6)
    xsc_full = (rmax / XQ).astype(np.float16)
    xq_full = np.clip(
        np.round(x / xsc_full.astype(np.float32)[:, None]), -127, 127
    ).astype(np.int8)

    percore = []
    for c in range(N_CORES):
        t0, t1 = int(cb[c]), int(cb[c + 1])
        ids = order[t0:t1]
        lseg = souts[t0:t1] - c * Sc
        kk = ks_full[ids]
        T = t1 - t0
        if T == 0:
            percore.append(None)
            continue
        is_start = np.r_[True, lseg[1:] != lseg[:-1]]
        B = np.flatnonzero(is_start)
        breaks = [0]
        while T - breaks[-1] > P:
            j = np.searchsorted(B, breaks[-1] + P, side="right") - 1
            nxt = int(B[j])
            assert nxt > breaks[-1], "segment longer than 128 tokens"
            breaks.append(nxt)
        breaks.append(T)
        breaks = np.asarray(breaks)
        nb = len(breaks) - 1
        tix = np.searchsorted(breaks, np.arange(T), side="right") - 1
        slot = np.arange(T) - breaks[tix]
        segfirst = lseg[breaks[:-1]]
        code = lseg - segfirst[tix]
        assert code.max() < P
        percore.append(
            {
                "ids": ids,
                "tix": tix,
                "slot": slot,
                "code": code,
                "kk": kk,
                "nb": nb,
                "segfirst": segfirst,
                "nseg": int(lseg[-1]) + 1,
            }
        )

    nt = max(pc["nb"] for pc in percore if pc is not None)
    cores = []
    metas = []
    for c in range(N_CORES):
        pc = percore[c]
        xs = np.zeros((nt * P, C), np.int8)
        xsc = np.zeros((P, nt), np.float16)
        codes = np.full((P, nt), 255, np.uint8)
        kcodes = np.zeros((P, nt), np.uint8)
        if pc is not None:
            g = pc["tix"] * P + pc["slot"]
            xs[g] = xq_full[pc["ids"]]
            xsc[pc["slot"], pc["tix"]] = xsc_full[pc["ids"]]
            codes[pc["slot"], pc["tix"]] = pc["code"]
            kcodes[pc["slot"], pc["tix"]] = pc["kk"]
        cores.append(
            {
                "xs": np.ascontiguousarray(xs.T),
                "xsc": xsc,
                "codes": codes,
                "kcodes": kcodes,
                "wall": wall,
            }
        )
        metas.append(
            None
            if pc is None
            else {"segfirst": pc["segfirst"], "nseg": pc["nseg"]}
        )
    meta = {"nt": nt, "M": M, "Sc": Sc, "core_meta": metas}
    return cores, meta


def build_bass(meta):
    nt = meta["nt"]
    nc = bacc.Bacc("TRN2")
    xs = nc.dram_tensor("xs", [C, nt * P], mybir.dt.int8, kind="ExternalInput")
    xscd = nc.dram_tensor("xsc", [P, nt], mybir.dt.float16, kind="ExternalInput")
    codes = nc.dram_tensor("codes", [P, nt], mybir.dt.uint8, kind="ExternalInput")
    kcodes = nc.dram_tensor("kcodes", [P, nt], mybir.dt.uint8, kind="ExternalInput")
    wall = nc.dram_tensor("wall", [C, K * C], mybir.dt.float32, kind="ExternalInput")
    yq = nc.dram_tensor("yq", [P, nt * C], mybir.dt.int8, kind="ExternalOutput")
    ysc = nc.dram_tensor("ysc", [P, nt], mybir.dt.float16, kind="ExternalOutput")

    eq = mybir.AluOpType.is_equal
    mul = mybir.AluOpType.mult

    with tile.TileContext(nc) as tc:
        with (
            tc.tile_pool(name="const", bufs=1) as constp,
            tc.tile_pool(name="xq", bufs=2) as xqp,
            tc.tile_pool(name="xb", bufs=2) as xbp,
            tc.tile_pool(name="xsc", bufs=2) as xscp,
            tc.tile_pool(name="ct", bufs=2) as ctp,
            tc.tile_pool(name="kt", bufs=2) as ktp,
            tc.tile_pool(name="pm", bufs=3) as pmp,
            tc.tile_pool(name="pf", bufs=3) as pfp,
            tc.tile_pool(name="pb", bufs=3) as pbp,
            tc.tile_pool(name="s", bufs=3) as sp,
            tc.tile_pool(name="r", bufs=3) as rp,
            tc.tile_pool(name="ysg", bufs=2) as ysgp,
            tc.tile_pool(name="ssg", bufs=2) as ssgp,
            tc.tile_pool(name="p8", bufs=2, space="PSUM") as p8p,
            tc.tile_pool(name="py", bufs=2, space="PSUM") as pyp,
        ):
            iota_s = constp.tile([P, P], mybir.dt.int16, tag="iota_s")
            nc.gpsimd.iota(iota_s[:], [[1, P]], channel_multiplier=0)
            iota_k = constp.tile([P, K * C], mybir.dt.int16, tag="iota_k")
            nc.gpsimd.iota(iota_k[:], [[1, K], [0, C]], channel_multiplier=0)
            wf = constp.tile([C, K * C], mybir.dt.float32, tag="wf")
            nc.sync.dma_start(out=wf[:], in_=wall[:, :])
            wsb = constp.tile([C, K * C], mybir.dt.bfloat16, tag="wsb")
            nc.vector.tensor_copy(out=wsb[:], in_=wf[:])

            nbatch = -(-nt // BATCH)
            for b in range(nbatch):
                j0 = b * BATCH
                nbt = min(BATCH, nt - j0)
                xqt = xqp.tile([C, nbt * P], mybir.dt.int8, tag="xq")
                nc.sync.dma_start(out=xqt[:], in_=xs[:, j0 * P : (j0 + nbt) * P])
                xbt = xbp.tile([C, nbt * P], mybir.dt.bfloat16, tag="xb")
                nc.vector.tensor_copy(out=xbt[:], in_=xqt[:])
                xsh = xscp.tile([P, nbt], mybir.dt.float16, tag="xsh")
                nc.sync.dma_start(out=xsh[:], in_=xscd[:, j0 : j0 + nbt])
                xsf = xscp.tile([P, nbt], mybir.dt.float32, tag="xsf")
                nc.vector.tensor_copy(out=xsf[:], in_=xsh[:])
                cti = ctp.tile([P, nbt], mybir.dt.uint8, tag="cti")
                nc.sync.dma_start(out=cti[:], in_=codes[:, j0 : j0 + nbt])
                ct = ctp.tile([P, nbt], mybir.dt.float32, tag="ct")
                nc.vector.tensor_copy(out=ct[:], in_=cti[:])
                kti = ktp.tile([P, nbt], mybir.dt.uint8, tag="kti")
                nc.sync.dma_start(out=kti[:], in_=kcodes[:, j0 : j0 + nbt])
                kt = ktp.tile([P, nbt], mybir.dt.float32, tag="kt")
                nc.vector.tensor_copy(out=kt[:], in_=kti[:])
                ysg = ysgp.tile([P, nbt * C], mybir.dt.int8, tag="ysg")
                ssg = ssgp.tile([P, nbt], mybir.dt.float16, tag="ssg")
                for j in range(nbt):
                    p8 = p8p.tile([P, K * C], mybir.dt.float32, tag="p8")
                    nc.tensor.matmul(
                        out=p8[:],
                        lhsT=xbt[:, j * P : (j + 1) * P],
                        rhs=wsb[:],
                        start=True,
                        stop=True,
                    )
                    pm = pmp.tile([P, K * C], mybir.dt.bfloat16, tag="pm")
                    nc.vector.scalar_tensor_tensor(
                        out=pm[:],
                        in0=iota_k[:],
                        scalar=kt[:, j : j + 1],
                        in1=p8[:],
                        op0=eq,
                        op1=mul,
                    )
                    pf = pfp.tile([P, C], mybir.dt.float32, tag="pf")
                    nc.vector.tensor_reduce(
                        out=pf[:],
                        in_=pm[:].rearrange("p (k c) -> p c k", k=K),
                        axis=mybir.AxisListType.X,
                        op=mybir.AluOpType.add,
                    )
                    pb = pbp.tile([P, C], mybir.dt.bfloat16, tag="pb")
                    nc.vector.tensor_single_scalar(
                        out=pb[:], in_=pf[:], scalar=xsf[:, j : j + 1], op=mul
                    )
                    s = sp.tile([P, P], mybir.dt.bfloat16, tag="s")
                    nc.gpsimd.tensor_scalar(
                        out=s[:],
                        in0=iota_s[:],
                        scalar1=ct[:, j : j + 1],
                        scalar2=None,
                        op0=eq,
                    )
                    y = pyp.tile([P, C], mybir.dt.float32, tag="y")
                    nc.tensor.matmul(
                        out=y[:], lhsT=s[:], rhs=pb[:], start=True, stop=True
                    )
                    rmax = rp.tile([P, 1], mybir.dt.float32, tag="rmax")
                    nc.vector.tensor_reduce(
                        out=rmax[:],
                        in_=y[:],
                        axis=mybir.AxisListType.X,
                        op=mybir.AluOpType.max,
                        apply_absolute_value=True,
                    )
                    rg = rp.tile([P, 1], mybir.dt.float32, tag="rg")
                    nc.vector.tensor_scalar_max(out=rg[:], in0=rmax[:], scalar1=1e-6)
                    inv = rp.tile([P, 1], mybir.dt.float32, tag="inv")
                    nc.vector.reciprocal(out=inv[:], in_=rg[:])
                    nc.vector.tensor_scalar(
                        out=ysg[:, j * C : (j + 1) * C],
                        in0=y[:],
                        scalar1=inv[:, 0:1],
                        scalar2=YQ,
                        op0=mul,
                        op1=mul,
                    )
                    nc.vector.tensor_scalar_mul(
                        out=ssg[:, j : j + 1], in0=rg[:], scalar1=1.0 / YQ
                    )
                nc.sync.dma_start(
                    out=yq[:, j0 * C : (j0 + nbt) * C], in_=ysg[:]
                )
                nc.sync.dma_start(out=ysc[:, j0 : j0 + nbt], in_=ssg[:])
    nc.compile()
    return nc


def assemble(results, meta, num_out):
    nt = meta["nt"]
    Sc = meta["Sc"]
    y = np.zeros((num_out, C), np.float32)
    for c in range(N_CORES):
        cm = meta["core_meta"][c]
        if cm is None:
            continue
        yqc = np.asarray(results[c]["yq"]).reshape(P, nt, C)
        yscc = np.asarray(results[c]["ysc"], np.float32)
        segfirst = cm["segfirst"]
        nseg = cm["nseg"]
        s = np.arange(nseg)
        tau = np.searchsorted(segfirst, s, side="right") - 1
        r = s - segfirst[tau]
        y[c * Sc : c * Sc + nseg] = (
            yqc[r, tau, :].astype(np.float32) * yscc[r, tau][:, None]
        )
    return y


def kernel(x, weight, offset_idx, out_idx, num_out):
    from concourse.bass_utils import run_bass_kernel_spmd

    num_out = int(num_out)
    cores, meta = host_prepare(x, weight, offset_idx, out_idx)
    nc = build_bass(meta)
    res = run_bass_kernel_spmd(nc, cores, core_ids=list(range(N_CORES)))
    return assemble(res.results, meta, num_out)


def _sim_test():
    """Small-scale correctness check in CoreSim (no hardware)."""
    from concourse.bass_interp import CoreSim

    rng = np.random.default_rng(0)
    N, GRID = 20000, 64
    coords = rng.integers(0, GRID, size=(N, 3))
    off = coords % 2
    offset_idx = (off[:, 0] * 4 + off[:, 1] * 2 + off[:, 2]).astype(np.int64)
    oc = coords // 2
    flat = (oc[:, 0] * (GRID // 2) + oc[:, 1]) * (GRID // 2) + oc[:, 2]
    _, inv = np.unique(flat, return_inverse=True)
    out_idx = inv.astype(np.int64)
    x = rng.standard_normal((N, C), np.float32)
    w = (rng.standard_normal((K, C, C)) * 0.1).astype(np.float32)

    M = int(out_idx.max()) + 1
    expected = np.zeros((M, C), np.float32)
    for k in range(K):
        sel = offset_idx == k
        np.add.at(expected, out_idx[sel], x[sel] @ w[k])

    cores, meta = host_prepare(x, w, offset_idx, out_idx)
    print(f"sim test: N={N} M={M} nt={meta['nt']}")
    nc = build_bass(meta)
    results = []
    for c in range(N_CORES):
        sim = CoreSim(nc, require_finite=False, require_nnan=False)
        for name, arr in cores[c].items():
            sim.tensor(name)[:] = arr
        sim.simulate(check_with_hw=False)
        results.append({"yq": sim.tensor("yq").copy(), "ysc": sim.tensor("ysc").copy()})
        print(f"core {c} simulated")
    actual = assemble(results, meta, M)
    denom = np.abs(expected).max()
    rel = np.abs(actual - expected).max() / denom
    print(f"sim relative error: {rel:.3e}")
    assert rel < 2e-2, rel


if __name__ == "__main__":
    _sim_test()


# revision 20
# speedup vs baseline: 1.5090x; 1.3216x over previous
"""Trainium2 Bass kernel for sparse 3D conv (gather -> 8x[32,32] GEMM -> scatter-add).

Design: tokens are sorted by output voxel id and partitioned across 8 cores by
contiguous output-row ranges, so each output row is owned by exactly one core
(no cross-core reduction, no RMW scatter). Per core, tokens are packed into
128-token tiles that never split a segment. On device, each tile does:

  P8 = X_tile^T @ W_all          one matmul, [128 tok, 8*32] for all 8 kernel offsets
  P  = reduce_k(onehot(k) * P8)  per-token offset select (DVE)
  S  = (iota == segcode)         one-hot token->segment matrix (DVE)
  Y  = S^T @ P                   segment sums, [128 seg-slots, 32] (matmul, PSUM)

Y rows are written densely per tile (unique rows, single write). Wire traffic is
minimized because the axon tunnel is the bottleneck (~55 MB/s incompressible):
x is uploaded as int8 with a global scale folded into the weights, y is
downloaded as int8 with per-row fp16 scales computed on device.
"""

import sys

sys.path.insert(0, "/opt/trn_rl_repo")

import numpy as np

import concourse.bacc as bacc
import concourse.mybir as mybir
import concourse.tile as tile
from concourse.bass import ds

P = 128  # tokens per tile == segment slots per tile
C = 32
K = 8
N_CORES = 8
BATCH = 16  # tiles per DMA group
XQ = 126.0  # int8 quant ranges (margin below 127 to avoid saturation)
YQ = 126.0


def host_prepare(x, weight, offset_idx, out_idx):
    x = np.asarray(x, np.float32)
    w = np.asarray(weight, np.float32)
    ks_full = np.asarray(offset_idx, np.int64)
    oi_full = np.asarray(out_idx, np.int64)
    N = x.shape[0]
    M = int(oi_full.max()) + 1
    Sc = -(-M // N_CORES)  # output rows per core

    order = np.argsort(oi_full, kind="stable")
    souts = oi_full[order]
    cb = np.searchsorted(souts, np.arange(N_CORES + 1) * Sc)

    wall = np.transpose(w, (1, 0, 2)).reshape(C, K * C).astype(np.float32)
    # per-token int8 quantization: x[i] ~= xq[i] * xsc[i]
    rmax = np.maximum(np.abs(x).max(axis=1), 1e-6)
    xsc_full = (rmax / XQ).astype(np.float16)
    xq_full = np.clip(
        np.round(x / xsc_full.astype(np.float32)[:, None]), -127, 127
    ).astype(np.int8)

    percore = []
    for c in range(N_CORES):
        t0, t1 = int(cb[c]), int(cb[c + 1])
        ids = order[t0:t1]
        lseg = souts[t0:t1] - c * Sc
        kk = ks_full[ids]
        T = t1 - t0
        if T == 0:
            percore.append(None)
            continue
        is_start = np.r_[True, lseg[1:] != lseg[:-1]]
        B = np.flatnonzero(is_start)
        breaks = [0]
        while T - breaks[-1] > P:
            j = np.searchsorted(B, breaks[-1] + P, side="right") - 1
            nxt = int(B[j])
            assert nxt > breaks[-1], "segment longer than 128 tokens"
            breaks.append(nxt)
        breaks.append(T)
        breaks = np.asarray(breaks)
        nb = len(breaks) - 1
        tix = np.searchsorted(breaks, np.arange(T), side="right") - 1
        slot = np.arange(T) - breaks[tix]
        segfirst = lseg[breaks[:-1]]
        code = lseg - segfirst[tix]
        assert code.max() < P
        percore.append(
            {
                "ids": ids,
                "tix": tix,
                "slot": slot,
                "code": code,
                "kk": kk,
                "nb": nb,
                "segfirst": segfirst,
                "nseg": int(lseg[-1]) + 1,
            }
        )

    nt = max(pc["nb"] for pc in percore if pc is not None)
    nt = -(-nt // BATCH) * BATCH  # pad to whole DMA batches for the For_i body
    nb = nt // BATCH
    cores = []
    metas = []
    for c in range(N_CORES):
        pc = percore[c]
        xs = np.zeros((nt * P, C), np.int8)
        xsc = np.zeros((P, nt), np.float16)
        codes = np.full((P, nt), 255, np.uint8)
        kcodes = np.zeros((P, nt), np.uint8)
        if pc is not None:
            g = pc["tix"] * P + pc["slot"]
            xs[g] = xq_full[pc["ids"]]
            xsc[pc["slot"], pc["tix"]] = xsc_full[pc["ids"]]
            codes[pc["slot"], pc["tix"]] = pc["code"]
            kcodes[pc["slot"], pc["tix"]] = pc["kk"]
        cores.append(
            {
                # batch-major layouts so the For_i body indexes every tensor
                # with the same ds(i, 1) leading-dim slice
                "xs": np.ascontiguousarray(
                    xs.reshape(nb, BATCH * P, C).transpose(0, 2, 1)
                ),
                "xsc": np.ascontiguousarray(
                    xsc.reshape(P, nb, BATCH).transpose(1, 0, 2)
                ),
                "codes": np.ascontiguousarray(
                    codes.reshape(P, nb, BATCH).transpose(1, 0, 2)
                ),
                "kcodes": np.ascontiguousarray(
                    kcodes.reshape(P, nb, BATCH).transpose(1, 0, 2)
                ),
                "wall": wall,
            }
        )
        metas.append(
            None
            if pc is None
            else {"segfirst": pc["segfirst"], "nseg": pc["nseg"]}
        )
    meta = {"nt": nt, "M": M, "Sc": Sc, "core_meta": metas}
    return cores, meta


def build_bass(meta):
    nt = meta["nt"]
    nb = nt // BATCH
    nc = bacc.Bacc("TRN2")
    xs = nc.dram_tensor(
        "xs", [nb, C, BATCH * P], mybir.dt.int8, kind="ExternalInput"
    )
    xscd = nc.dram_tensor(
        "xsc", [nb, P, BATCH], mybir.dt.float16, kind="ExternalInput"
    )
    codes = nc.dram_tensor("codes", [nb, P, BATCH], mybir.dt.uint8, kind="ExternalInput")
    kcodes = nc.dram_tensor(
        "kcodes", [nb, P, BATCH], mybir.dt.uint8, kind="ExternalInput"
    )
    wall = nc.dram_tensor("wall", [C, K * C], mybir.dt.float32, kind="ExternalInput")
    yq = nc.dram_tensor("yq", [nb, P, BATCH * C], mybir.dt.int8, kind="ExternalOutput")
    ysc = nc.dram_tensor("ysc", [nb, P, BATCH], mybir.dt.float16, kind="ExternalOutput")

    def sq(ap):
        return ap.rearrange("o p f -> (o p) f")

    eq = mybir.AluOpType.is_equal
    mul = mybir.AluOpType.mult

    with tile.TileContext(nc) as tc:
        with (
            tc.tile_pool(name="const", bufs=1) as constp,
            tc.tile_pool(name="xq", bufs=2) as xqp,
            tc.tile_pool(name="xb", bufs=2) as xbp,
            tc.tile_pool(name="xsc", bufs=2) as xscp,
            tc.tile_pool(name="ct", bufs=2) as ctp,
            tc.tile_pool(name="kt", bufs=2) as ktp,
            tc.tile_pool(name="pm", bufs=3) as pmp,
            tc.tile_pool(name="pf", bufs=3) as pfp,
            tc.tile_pool(name="pb", bufs=3) as pbp,
            tc.tile_pool(name="s", bufs=3) as sp,
            tc.tile_pool(name="r", bufs=3) as rp,
            tc.tile_pool(name="ysg", bufs=2) as ysgp,
            tc.tile_pool(name="ssg", bufs=2) as ssgp,
            tc.tile_pool(name="p8", bufs=2, space="PSUM") as p8p,
            tc.tile_pool(name="py", bufs=2, space="PSUM") as pyp,
        ):
            iota_s = constp.tile([P, P], mybir.dt.int16, tag="iota_s")
            nc.gpsimd.iota(iota_s[:], [[1, P]], channel_multiplier=0)
            iota_k = constp.tile([P, K * C], mybir.dt.int16, tag="iota_k")
            nc.gpsimd.iota(iota_k[:], [[1, K], [0, C]], channel_multiplier=0)
            wf = constp.tile([C, K * C], mybir.dt.float32, tag="wf")
            nc.sync.dma_start(out=wf[:], in_=wall[:, :])
            wsb = constp.tile([C, K * C], mybir.dt.bfloat16, tag="wsb")
            nc.vector.tensor_copy(out=wsb[:], in_=wf[:])

            with tc.For_i(0, nb, 1) as bi:
                nbt = BATCH
                xqt = xqp.tile([C, nbt * P], mybir.dt.int8, tag="xq")
                nc.sync.dma_start(out=xqt[:], in_=sq(xs[ds(bi, 1)]))
                xbt = xbp.tile([C, nbt * P], mybir.dt.bfloat16, tag="xb")
                nc.vector.tensor_copy(out=xbt[:], in_=xqt[:])
                xsh = xscp.tile([P, nbt], mybir.dt.float16, tag="xsh")
                nc.sync.dma_start(out=xsh[:], in_=sq(xscd[ds(bi, 1)]))
                xsf = xscp.tile([P, nbt], mybir.dt.float32, tag="xsf")
                nc.vector.tensor_copy(out=xsf[:], in_=xsh[:])
                cti = ctp.tile([P, nbt], mybir.dt.uint8, tag="cti")
                nc.sync.dma_start(out=cti[:], in_=sq(codes[ds(bi, 1)]))
                ct = ctp.tile([P, nbt], mybir.dt.float32, tag="ct")
                nc.vector.tensor_copy(out=ct[:], in_=cti[:])
                kti = ktp.tile([P, nbt], mybir.dt.uint8, tag="kti")
                nc.sync.dma_start(out=kti[:], in_=sq(kcodes[ds(bi, 1)]))
                kt = ktp.tile([P, nbt], mybir.dt.float32, tag="kt")
                nc.vector.tensor_copy(out=kt[:], in_=kti[:])
                ysg = ysgp.tile([P, nbt * C], mybir.dt.int8, tag="ysg")
                ssg = ssgp.tile([P, nbt], mybir.dt.float16, tag="ssg")
                for j in range(nbt):
                    p8 = p8p.tile([P, K * C], mybir.dt.float32, tag="p8")
                    nc.tensor.matmul(
                        out=p8[:],
                        lhsT=xbt[:, j * P : (j + 1) * P],
                        rhs=wsb[:],
                        start=True,
                        stop=True,
                    )
                    pm = pmp.tile([P, K * C], mybir.dt.bfloat16, tag="pm")
                    nc.vector.scalar_tensor_tensor(
                        out=pm[:],
                        in0=iota_k[:],
                        scalar=kt[:, j : j + 1],
                        in1=p8[:],
                        op0=eq,
                        op1=mul,
                    )
                    pf = pfp.tile([P, C], mybir.dt.float32, tag="pf")
                    nc.vector.tensor_reduce(
                        out=pf[:],
                        in_=pm[:].rearrange("p (k c) -> p c k", k=K),
                        axis=mybir.AxisListType.X,
                        op=mybir.AluOpType.add,
                    )
                    pb = pbp.tile([P, C], mybir.dt.bfloat16, tag="pb")
                    nc.vector.tensor_single_scalar(
                        out=pb[:], in_=pf[:], scalar=xsf[:, j : j + 1], op=mul
                    )
                    s = sp.tile([P, P], mybir.dt.bfloat16, tag="s")
                    nc.gpsimd.tensor_scalar(
                        out=s[:],
                        in0=iota_s[:],
                        scalar1=ct[:, j : j + 1],
                        scalar2=None,
                        op0=eq,
                    )
                    y = pyp.tile([P, C], mybir.dt.float32, tag="y")
                    nc.tensor.matmul(
                        out=y[:], lhsT=s[:], rhs=pb[:], start=True, stop=True
                    )
                    rmax = rp.tile([P, 1], mybir.dt.float32, tag="rmax")
                    nc.vector.tensor_reduce(
                        out=rmax[:],
                        in_=y[:],
                        axis=mybir.AxisListType.X,
                        op=mybir.AluOpType.max,
                        apply_absolute_value=True,
                    )
                    rg = rp.tile([P, 1], mybir.dt.float32, tag="rg")
                    nc.vector.tensor_scalar_max(out=rg[:], in0=rmax[:], scalar1=1e-6)
                    inv = rp.tile([P, 1], mybir.dt.float32, tag="inv")
                    nc.vector.reciprocal(out=inv[:], in_=rg[:])
                    nc.vector.tensor_scalar(
                        out=ysg[:, j * C : (j + 1) * C],
                        in0=y[:],
                        scalar1=inv[:, 0:1],
                        scalar2=YQ,
                        op0=mul,
                        op1=mul,
                    )
                    nc.vector.tensor_scalar_mul(
                        out=ssg[:, j : j + 1], in0=rg[:], scalar1=1.0 / YQ
                    )
                nc.sync.dma_start(out=sq(yq[ds(bi, 1)]), in_=ysg[:])
                nc.sync.dma_start(out=sq(ysc[ds(bi, 1)]), in_=ssg[:])
    nc.compile()
    return nc


def assemble(results, meta, num_out):
    nt = meta["nt"]
    Sc = meta["Sc"]
    y = np.zeros((num_out, C), np.float32)
    for c in range(N_CORES):
        cm = meta["core_meta"][c]
        if cm is None:
            continue
        nb = nt // BATCH
        yqc = (
            np.asarray(results[c]["yq"])
            .reshape(nb, P, BATCH, C)
            .transpose(1, 0, 2, 3)
            .reshape(P, nt, C)
        )
        yscc = (
            np.asarray(results[c]["ysc"], np.float32)
            .reshape(nb, P, BATCH)
            .transpose(1, 0, 2)
            .reshape(P, nt)
        )
        segfirst = cm["segfirst"]
        nseg = cm["nseg"]
        s = np.arange(nseg)
        tau = np.searchsorted(segfirst, s, side="right") - 1
        r = s - segfirst[tau]
        y[c * Sc : c * Sc + nseg] = (
            yqc[r, tau, :].astype(np.float32) * yscc[r, tau][:, None]
        )
    return y


def kernel(x, weight, offset_idx, out_idx, num_out):
    from concourse.bass_utils import run_bass_kernel_spmd

    num_out = int(num_out)
    cores, meta = host_prepare(x, weight, offset_idx, out_idx)
    nc = build_bass(meta)
    res = run_bass_kernel_spmd(nc, cores, core_ids=list(range(N_CORES)))
    return assemble(res.results, meta, num_out)


def _sim_test():
    """Small-scale correctness check in CoreSim (no hardware)."""
    from concourse.bass_interp import CoreSim

    rng = np.random.default_rng(0)
    N, GRID = 20000, 64
    coords = rng.integers(0, GRID, size=(N, 3))
    off = coords % 2
    offset_idx = (off[:, 0] * 4 + off[:, 1] * 2 + off[:, 2]).astype(np.int64)
    oc = coords // 2
    flat = (oc[:, 0] * (GRID // 2) + oc[:, 1]) * (GRID // 2) + oc[:, 2]
    _, inv = np.unique(flat, return_inverse=True)
    out_idx = inv.astype(np.int64)
    x = rng.standard_normal((N, C), np.float32)
    w = (rng.standard_normal((K, C, C)) * 0.1).astype(np.float32)

    M = int(out_idx.max()) + 1
    expected = np.zeros((M, C), np.float32)
    for k in range(K):
        sel = offset_idx == k
        np.add.at(expected, out_idx[sel], x[sel] @ w[k])

    cores, meta = host_prepare(x, w, offset_idx, out_idx)
    print(f"sim test: N={N} M={M} nt={meta['nt']}")
    nc = build_bass(meta)
    results = []
    for c in range(N_CORES):
        sim = CoreSim(nc, require_finite=False, require_nnan=False)
        for name, arr in cores[c].items():
            sim.tensor(name)[:] = arr
        sim.simulate(check_with_hw=False)
        results.append({"yq": sim.tensor("yq").copy(), "ysc": sim.tensor("ysc").copy()})
        print(f"core {c} simulated")
    actual = assemble(results, meta, M)
    denom = np.abs(expected).max()
    rel = np.abs(actual - expected).max() / denom
    print(f"sim relative error: {rel:.3e}")
    assert rel < 2e-2, rel


if __name__ == "__main__":
    _sim_test()
